# revision 40
# baseline (speedup 1.0000x reference)
"""Grouped-Query Attention (B=2, T=2048, E=2048, 16 Q heads / 4 KV heads, RoPE,
causal) as a Bass/Tile kernel on 8 Trainium2 NeuronCores.

Sharding: core c = 4*b + h handles batch b (of 2) and KV-head group h (of 4,
i.e. 4 q-heads + 1 kv head).  Each core computes its QKV projections (channel
sharded), RoPE, causal attention for its 4 q heads, and a partial
out-projection over its 512 channels of Wo.  The 4 partial out-proj results per
batch are summed on the host during unsharding (row-sharded out_proj).

All matmul operands are bf16 (fp32 accumulation in PSUM); scores use ragged
causal widths (per 128-token k-tile) so masked work is skipped; softmax
denominators come from one ones-matmul over a DVE/GPSIMD-accumulated exp sum
per 512-token q chunk instead of per k-tile; stationary operands are reused
across wide moving streams to minimize LDWEIGHTS; projection of head h+1 is
interleaved into attention of head h to keep the PE array hot.
"""

import os
import sys

import numpy as np

try:
    import concourse.bass as bass
except ModuleNotFoundError:  # fresh grading dir: point at the in-container repo
    for p in ("/opt/trn_rl_repo", "/root/.axon_site/_ro/trn_rl_repo"):
        if os.path.isdir(p) and p not in sys.path:
            sys.path.insert(0, p)
    import concourse.bass as bass

from contextlib import ExitStack

from ml_dtypes import bfloat16

import concourse.tile as tile
from concourse import bacc, mybir
from concourse.bass_utils import run_bass_kernel_spmd

# ---- problem constants (hardcoded per contract) ----
B, T, E = 2, 2048, 2048
N_QHEAD, N_KVHEAD = 16, 4
GROUP = N_QHEAD // N_KVHEAD          # 4 q heads per kv head
D = E // N_QHEAD                     # 128 head dim
KV = E // GROUP                      # 512 kv dim
ROPE_BASE = 10000.0
N_CORES = 8

P = 128                              # partitions
ET = E // P                          # 16 e-tiles
TT = T // P                          # 16 token tiles
TC = 512                             # q-chunk / PSUM-bank width (fp32)
NTC = T // TC                        # 4 chunks

F32 = mybir.dt.float32
F32R = mybir.dt.float32r
BF16 = mybir.dt.bfloat16

_CACHE = {}


def _build_program():
    """Build + compile the (SPMD-identical) Bass program once per process."""
    if "nc" in _CACHE:
        return _CACHE["nc"]

    nc = bacc.Bacc("TRN2", target_bir_lowering=False, debug=False,
                   num_devices=N_CORES)

    dram = {}
    def din(name, shape, dt=BF16):
        dram[name] = nc.dram_tensor(name, list(shape), dt,
                                    kind="ExternalInput").ap()
    din("xt", (NTC, ET, P, TC))         # x[b].T: (tq-chunk, e-tile, p, t)
    din("wq", (P, GROUP * ET * P))      # WqT tiles, (ct, e) major
    din("wk", (P, ET * P))
    din("wv", (P, ET * P))
    din("wo", (P, GROUP * TT * P))      # WoS.T tiles (ct, jt) major
    din("bias6", (P, 6), F32)           # per-ctile biases: 4x bq, k, v
    din("bo16", (P, TT), F32)           # bo per j-tile (zeros on h!=0 cores)
    din("cosq", (P, T))                 # rope tables, q scaled 1/sqrt(D)
    din("sinq", (P, T))
    din("cosk", (P, T))
    din("sink", (P, T))
    din("ptm", (P, P))                  # rope rotation matrix Pm^T (lhsT)
    din("ident", (P, P))                # identity (for PE transpose)
    din("maskd", (P, P))                # diagonal-block causal mask (tq>=tk)
    din("onescol", (P, GROUP * P))      # tile qc: ones in column qc
    din("selq", (NTC, GROUP * P))       # tile qc: ones in row qc
    outt = nc.dram_tensor("outt", [TT, P, T], BF16, kind="ExternalOutput").ap()

    with tile.TileContext(nc) as tc:
        with ExitStack() as ctx, nc.allow_low_precision(
                reason="bf16 matmul operands; accumulation stays fp32 in PSUM"):
            persist = ctx.enter_context(tc.tile_pool(name="persist", bufs=1))

            def ptile(shape, name, dt=BF16):
                return persist.tile(shape, dt, tag=name, name=name)

            # ---------- persistent SBUF tiles ----------
            wq_sb = ptile([P, GROUP * ET * P], "wq_sb")
            wk_sb = ptile([P, ET * P], "wk_sb")
            wv_sb = ptile([P, ET * P], "wv_sb")
            wo_sb = ptile([P, GROUP * TT * P], "wo_sb")
            bias6_sb = ptile([P, 8], "bias6_sb", F32)
            bo16_sb = ptile([P, TT], "bo16_sb", F32)
            cosq_sb = ptile([P, T], "cosq_sb")
            sinq_sb = ptile([P, T], "sinq_sb")
            cosk_sb = ptile([P, T], "cosk_sb")
            sink_sb = ptile([P, T], "sink_sb")
            ptm_sb = ptile([P, P], "ptm_sb")
            ident_sb = ptile([P, P], "ident_sb")
            maskd_sb = ptile([P, P], "maskd_sb")
            onescol_sb = ptile([P, GROUP * P], "onescol_sb")
            selq_sb = ptile([NTC, GROUP * P], "selq_sb")
            x_sb = ptile([P, NTC * ET * TC], "x_sb")     # full xT, resident
            kT_sb = ptile([P, T], "kT_sb")
            vT_sb = ptile([P, T], "vT_sb")
            vtok_sb = ptile([P, T], "vtok_sb")
            qA_sb = ptile([P, T], "qA_sb")               # qT, heads alternate
            qB_sb = ptile([P, T], "qB_sb")
            y_sb = ptile([P, GROUP * T], "y_sb")

            # SBUF pools
            egp = ctx.enter_context(tc.tile_pool(name="egp", bufs=3))
            osb = ctx.enter_context(tc.tile_pool(name="osb", bufs=2))
            nsb = ctx.enter_context(tc.tile_pool(name="nsb", bufs=2))
            # PSUM pools: 4 + 2 + 1 + 1 = 8 banks
            pyps = ctx.enter_context(tc.tile_pool(name="pyps", bufs=4, space="PSUM"))
            pscr = ctx.enter_context(tc.tile_pool(name="pscr", bufs=2, space="PSUM"))
            pmisc = ctx.enter_context(tc.tile_pool(name="pmisc", bufs=1, space="PSUM"))
            pden = ctx.enter_context(tc.tile_pool(name="pden", bufs=1, space="PSUM"))

            # ---------- load constants (ordered by first use) ----------
            x3 = x_sb[:].rearrange("p (c e t) -> p c e t", c=NTC, e=ET)

            def dma_x(xc, nsplit=4):
                xd = dram["xt"][xc].rearrange("e p t -> p e t")
                step = ET // nsplit
                for q in range(nsplit):
                    nc.sync.dma_start(x3[:, xc, q * step:(q + 1) * step, :],
                                      xd[:, q * step:(q + 1) * step, :])

            for nm, t in [("wk", wk_sb), ("wv", wv_sb), ("bias6", bias6_sb)]:
                if nm == "bias6":
                    nc.sync.dma_start(t[:, 0:6], dram[nm][:])
                else:
                    nc.sync.dma_start(t[:], dram[nm][:])
            dma_x(0, nsplit=8)
            nc.sync.dma_start(wq_sb[:, 0:ET * P], dram["wq"][:, 0:ET * P])
            for nm, t in [("ptm", ptm_sb), ("ident", ident_sb),
                          ("maskd", maskd_sb), ("onescol", onescol_sb),
                          ("selq", selq_sb)]:
                nc.sync.dma_start(t[:], dram[nm][:])

            def wq_tile(ct, e):
                return wq_sb[:, (ct * ET + e) * P:(ct * ET + e + 1) * P]

            def proj_mms(pool, tag, lhs_of_e, xc, e0, e1, psum_holder):
                """Emit e-mms [e0,e1) accumulating over e into psum_holder."""
                if e0 == 0:
                    psum_holder[0] = pool.tile([P, TC], F32, tag=tag,
                                               name="pacc")
                pp = psum_holder[0]
                for e in range(e0, e1):
                    nc.tensor.matmul(pp[:], lhs_of_e(e),
                                     x_sb[:, (xc * ET + e) * TC:
                                          (xc * ET + e + 1) * TC],
                                     start=(e == 0), stop=(e == ET - 1))

            def proj_evac(dst, psum_holder, bias_col):
                nc.vector.tensor_scalar_add(dst, psum_holder[0][:],
                                            bias6_sb[:, bias_col:bias_col + 1])

            # ---------- phase 1: k, v, q0 projections ----------
            # (wq ct=0 tile offset equals the wk/wv-style e*P slice)
            hold = [None]
            for xc in range(NTC):
                if xc > 0:
                    dma_x(xc)
                cs = slice(xc * TC, (xc + 1) * TC)
                for wsb, bcol, dst in ((wk_sb, 4, kT_sb), (wv_sb, 5, vT_sb),
                                       (wq_sb, 0, qA_sb)):
                    proj_mms(pyps, "acc",
                             lambda e, w=wsb: w[:, e * P:(e + 1) * P],
                             xc, 0, ET, hold)
                    proj_evac(dst[:, cs], hold, bcol)

            # tables for rope; rest of wq; wo prefetch
            nc.sync.dma_start(cosk_sb[:], dram["cosk"][:])
            nc.sync.dma_start(sink_sb[:], dram["sink"][:])
            nc.sync.dma_start(cosq_sb[:], dram["cosq"][:])
            nc.sync.dma_start(sinq_sb[:], dram["sinq"][:])
            for ct in range(1, GROUP):
                nc.sync.dma_start(wq_sb[:, ct * ET * P:(ct + 1) * ET * P],
                                  dram["wq"][:, ct * ET * P:(ct + 1) * ET * P])
            for ct in range(GROUP):
                nc.sync.dma_start(wo_sb[:, ct * TT * P:(ct + 1) * TT * P],
                                  dram["wo"][:, ct * TT * P:(ct + 1) * TT * P])
            nc.sync.dma_start(bo16_sb[:], dram["bo16"][:])

            # ---------- rope (4 chunks of 512; 3 DVE ops each) ----------
            def rope_chunk(dst_full, cos_sb, sin_sb, c):
                cs = slice(c * TC, (c + 1) * TC)
                rot = pscr.tile([P, TC], F32, tag="sc", name="rot")
                nc.tensor.matmul(rot[:], ptm_sb[:], dst_full[:, cs],
                                 start=True, stop=True)
                tmp = nsb.tile([P, TC], BF16, tag="rt", name="tmp")
                nc.vector.tensor_mul(tmp[:], rot[:], sin_sb[:, cs])
                nc.vector.tensor_mul(dst_full[:, cs], dst_full[:, cs],
                                     cos_sb[:, cs])
                nc.vector.tensor_add(dst_full[:, cs], dst_full[:, cs],
                                     tmp[:])

            # ---------- v -> token-major via PE transpose; rope k/q0 ----------
            # (vtok PE transposes fill the PE while the rope DVE ops run)
            for j in range(TT):
                vps = pscr.tile([P, P], BF16, tag="sc", name="vps")
                nc.tensor.transpose(vps[:], vT_sb[:, j * P:(j + 1) * P],
                                    ident_sb[:])
                nc.scalar.copy(vtok_sb[:, j * P:(j + 1) * P], vps[:])
            for c in range(NTC):
                rope_chunk(kT_sb[:], cosk_sb, sink_sb, c)
                rope_chunk(qA_sb[:], cosq_sb, sinq_sb, c)

            # ---------- phase 2: attention per head ----------
            # scores^T s[tk,tq] per k-tile j with ragged causal width
            # (tq >= 128j); exp on scalar; softmax denominators accumulate in
            # one PSUM bank ([NTC,TC], row per q-chunk) via per-(j,qc)
            # ones-matmuls right behind the AV matmuls. Scores of k-tile j+1
            # are emitted before consuming j so the PE never waits on exp.
            # Projection of head h+1 is interleaved at odd j.
            for h in range(GROUP):
                qT = qA_sb if h % 2 == 0 else qB_sb
                qN = qB_sb if h % 2 == 0 else qA_sb
                yps = [None] * NTC
                den_ps = pden.tile([P, TC], F32, tag="dn", name="den_ps")
                egs = [None] * TT
                phold = [None]
                pxc = [0]

                def next_proj():
                    # emit the next 1/8th of head h+1's projection
                    if h >= GROUP - 1 or pxc[0] >= 8:
                        return
                    xc, half = divmod(pxc[0], 2)
                    proj_mms(pmisc, "mi", lambda e: wq_tile(h + 1, e), xc,
                             half * 8, (half + 1) * 8, phold)
                    if half == 1:
                        proj_evac(qN[:, xc * TC:(xc + 1) * TC], phold, h + 1)
                    pxc[0] += 1

                def emit_scores(j):
                    w = T - j * P
                    base = j * P
                    eg = egp.tile([P, T], BF16, tag="eg", name="eg")
                    egs[j] = eg
                    for c0 in range(0, w, TC):
                        cw = min(TC, w - c0)
                        sps = pscr.tile([P, TC], F32, tag="sc", name="sps")
                        nc.tensor.matmul(
                            sps[:, 0:cw], kT_sb[:, j * P:(j + 1) * P],
                            qT[:, base + c0:base + c0 + cw],
                            start=True, stop=True)
                        nc.scalar.activation(
                            eg[:, c0:c0 + cw], sps[:, 0:cw],
                            mybir.ActivationFunctionType.Exp)

                emit_scores(0)
                for j in range(TT):
                    if j < TT - 1:
                        emit_scores(j + 1)
                    base = j * P
                    eg = egs[j]
                    # mask the diagonal 128-block
                    nc.vector.tensor_mul(eg[:, 0:P], eg[:, 0:P], maskd_sb[:])
                    # AV per overlapping q chunk (vtok_j stationary shared),
                    # then denominator colsums (ones-at-column-qc stationary
                    # puts each colsum in row qc of the shared den bank)
                    segs = []
                    for qc in range(j // GROUP, NTC):
                        s0 = max(qc * TC, base)
                        s1 = qc * TC + TC
                        segs.append((qc, s0, s1))
                        if j == 0:
                            yps[qc] = pyps.tile([P, TC], F32, tag="acc",
                                                name="yps")
                        nc.tensor.matmul(
                            yps[qc][:, s0 - qc * TC:s1 - qc * TC],
                            vtok_sb[:, j * P:(j + 1) * P],
                            eg[:, s0 - base:s1 - base],
                            start=(j == 0), stop=(j == GROUP * qc + GROUP - 1))
                        if j == GROUP * qc + GROUP - 1:
                            # evacuate unnormalized: frees the PSUM bank and
                            # keeps the reciprocal off the PE critical path
                            nc.vector.tensor_copy(
                                y_sb[:, h * T + qc * TC:h * T + (qc + 1) * TC],
                                yps[qc][:])
                    for qc, s0, s1 in segs:
                        nc.tensor.matmul(
                            den_ps[:, s0 - qc * TC:s1 - qc * TC],
                            onescol_sb[:, qc * P:(qc + 1) * P],
                            eg[:, s0 - base:s1 - base],
                            start=(j == 0 and qc == 0),
                            stop=(j == TT - 1 and qc == NTC - 1),
                            skip_group_check=True)
                    if j % 2 == 1:
                        next_proj()

                # head end. Emission order matters: rope chunk 0 of the next
                # head's q first (gates its scores), then the den hop to SBUF
                # (frees the den bank), then the rest of rope, then the
                # normalize chain (reciprocal + selector-broadcast + in-place
                # mul) which only outproj depends on.
                if h < GROUP - 1:
                    rope_chunk(qN[:], cosq_sb, sinq_sb, 0)
                den_sb = nsb.tile([NTC, TC], F32, tag="dn", name="den_sb")
                nc.vector.tensor_copy(den_sb[:], den_ps[0:NTC, :])
                if h < GROUP - 1:
                    for c in range(1, NTC):
                        rope_chunk(qN[:], cosq_sb, sinq_sb, c)
                rec_sb = nsb.tile([NTC, TC], BF16, tag="rc", name="rec_sb")
                nc.vector.reciprocal(rec_sb[:], den_sb[:])
                for qc in range(NTC):
                    bcs = pmisc.tile([P, TC], F32, tag="mi", name="bcs")
                    nc.tensor.matmul(bcs[:],
                                     selq_sb[:, qc * P:(qc + 1) * P],
                                     rec_sb[:], start=True, stop=True)
                    ys = y_sb[:, h * T + qc * TC:h * T + (qc + 1) * TC]
                    nc.vector.tensor_mul(ys, ys, bcs[:])

            # ---------- phase 3: partial out-projection ----------
            # outT[j,t] = sum_c WoST[c,j] * yT[c,t]   (+bo on core h==0);
            # scalar evacuates with the bias fused; one DMA per j-tile.
            for jt in range(TT):
                ops = [None] * NTC
                for ct in range(GROUP):
                    lhs = wo_sb[:, (ct * TT + jt) * P:(ct * TT + jt + 1) * P]
                    for c in range(NTC):
                        if ct == 0:
                            # alternate PSUM pools so two j-tiles pipeline
                            if jt % 2 == 0:
                                ops[c] = pyps.tile([P, TC], F32, tag="acc",
                                                   name="ops")
                            else:
                                pool, tg = ((pscr, "sc"), (pscr, "sc"),
                                            (pmisc, "mi"), (pden, "dn"))[c]
                                ops[c] = pool.tile([P, TC], F32, tag=tg,
                                                   name="ops")
                        nc.tensor.matmul(
                            ops[c][:], lhs,
                            y_sb[:, ct * T + c * TC:ct * T + (c + 1) * TC],
                            start=(ct == 0), stop=(ct == GROUP - 1))
                ostj = osb.tile([P, T], BF16, tag="ost", name="ostj")
                for c in range(NTC):
                    nc.scalar.activation(
                        ostj[:, c * TC:(c + 1) * TC], ops[c][:],
                        mybir.ActivationFunctionType.Identity,
                        bias=bo16_sb[:, jt:jt + 1])
                nc.sync.dma_start(outt[jt][:], ostj[:])

    nc.compile()
    _CACHE["nc"] = nc
    return nc


def _host_inputs(x, Wq, bq, Wk, bk, Wv, bv, Wo, bo):
    """Per-core input dicts (bf16 layouts matching the DRAM decls)."""
    f = np.float32
    i = np.arange(1, D // 2 + 1, dtype=np.float64)
    thetas = ROPE_BASE ** (-2.0 * (i - 1.0) / D)
    ang = np.arange(1, T + 1, dtype=np.float64)[:, None] * thetas      # [T, D/2]
    cos = np.concatenate([np.cos(ang), np.cos(ang)], axis=1).T
    sin = np.concatenate([np.sin(ang), np.sin(ang)], axis=1).T
    s = 1.0 / np.sqrt(D)
    cosq = np.ascontiguousarray((cos * s).astype(bfloat16))
    sinq = np.ascontiguousarray((sin * s).astype(bfloat16))
    cosk = np.ascontiguousarray(cos.astype(bfloat16))
    sink = np.ascontiguousarray(sin.astype(bfloat16))

    Pm = np.zeros((D, D), f)
    for d in range(D // 2):
        Pm[d, d + D // 2] = -1.0
        Pm[d + D // 2, d] = 1.0
    ptm = np.ascontiguousarray(Pm.T.astype(bfloat16))
    ident = np.eye(P, dtype=bfloat16)

    pcol = np.arange(P)[:, None]
    fcol = np.arange(P)[None, :]
    maskd = np.ascontiguousarray((pcol <= fcol).astype(bfloat16))
    onescol = np.zeros((P, GROUP * P), dtype=bfloat16)
    selq = np.zeros((NTC, GROUP * P), dtype=bfloat16)
    for qc in range(GROUP):
        onescol[:, qc * P + qc] = bfloat16(1.0)
        selq[qc, qc * P:(qc + 1) * P] = bfloat16(1.0)

    # xT per batch, chunked contiguous: (NTC, ET, P, TC)
    xts = []
    for b in range(B):
        xb = x[b].astype(bfloat16)                                     # [T, E]
        xt = np.ascontiguousarray(
            xb.T.reshape(ET, P, NTC, TC).transpose(2, 0, 1, 3))
        xts.append(xt)

    per_core = []
    for c in range(N_CORES):
        b, h = divmod(c, GROUP)
        WqS = Wq[h * KV:(h + 1) * KV, :]                               # [512, E]
        wq = np.ascontiguousarray(
            WqS.T.reshape(ET, P, GROUP, P).transpose(1, 2, 0, 3)
            .reshape(P, -1).astype(bfloat16))
        WkS = Wk[h * D:(h + 1) * D, :]
        wk = np.ascontiguousarray(
            WkS.T.reshape(ET, P, P).transpose(1, 0, 2).reshape(P, -1)
            .astype(bfloat16))
        WvS = Wv[h * D:(h + 1) * D, :]
        wv = np.ascontiguousarray(
            WvS.T.reshape(ET, P, P).transpose(1, 0, 2).reshape(P, -1)
            .astype(bfloat16))
        WoS = Wo[:, h * KV:(h + 1) * KV]                               # [E, 512]
        wo = np.ascontiguousarray(
            WoS.T.reshape(GROUP, P, TT, P).transpose(1, 0, 2, 3).reshape(P, -1)
            .astype(bfloat16))
        bias6 = np.stack([bq[h * KV + ct * P: h * KV + (ct + 1) * P]
                          for ct in range(GROUP)]
                         + [bk[h * D:(h + 1) * D], bv[h * D:(h + 1) * D]],
                         axis=1).astype(f)
        bo16 = (bo.reshape(TT, P).T if h == 0
                else np.zeros((P, TT), f)).astype(f)
        per_core.append({
            "xt": xts[b], "wq": wq, "wk": wk, "wv": wv, "wo": wo,
            "bias6": np.ascontiguousarray(bias6),
            "bo16": np.ascontiguousarray(bo16),
            "cosq": cosq, "sinq": sinq, "cosk": cosk, "sink": sink,
            "ptm": ptm, "ident": ident, "maskd": maskd,
            "onescol": onescol, "selq": selq,
        })
    return per_core


def kernel(**inputs):
    x = np.asarray(inputs["x"], np.float32)
    nc = _build_program()
    in_maps = _host_inputs(
        x, *(np.asarray(inputs[k], np.float32)
             for k in ("Wq", "bq", "Wk", "bk", "Wv", "bv", "Wo", "bo")))
    res = run_bass_kernel_spmd(nc, in_maps, list(range(N_CORES)))
    out = np.empty((B, T, E), np.float32)
    for b in range(B):
        acc = np.zeros((E, T), np.float32)
        for h in range(GROUP):
            acc += res.results[b * GROUP + h]["outt"].reshape(E, T)
        out[b] = acc.T
    return out
